# revision 1
# baseline (speedup 1.0000x reference)
"""Trainium2 Bass kernel for nn_Attention_53334903882008 (additive attention), v9.

Reference (per batch b):
  We  = img @ W^T + Wb;  Ue = (hid @ U^T + Ub) broadcast over T
  att = tanh(We + Ue);   e = att @ w + wb
  alpha = softmax_N(e);  phi = sum_n alpha * img      -> [B, T, D]

Sharding: data-parallel over B=8, one batch per NeuronCore; weights
replicated.

Dataflow (v3): all layout work is host-side. x is sent twice in bf16 —
xt ([d, btn] tiles, stationary operand of the We matmuls) and xn
([btn, d], moving operand of the phi matmuls) — no on-chip transposes,
no cast DMAs. Same 32 MB HBM traffic as one f32 copy.

Per 128-row btn-tile (64 tiles):
  - We[btn, h] = sum_kt xt_tile^T @ wt  (8 matmuls, PSUM f32)
  - + U_comb (DVE add in-place in PSUM), tanh on ACT -> att bf16
  - e column [128,1] via one fused DVE scalar_tensor_tensor
    (out = att * w_rep, accum_out = sum_h) — no PE matmul for e
  - adiag = exp(base_log + e) in ONE ACT op (bias = e per-partition;
    base_log is 0 on the block-diag band, -30000 off-band) -> the
    block-diagonal unnormalized-softmax matrix directly
  - phi[t,d] += adiag^T @ xn (2 matmuls) and s[t] += adiag^T @ ones
    (N=1 matmul) accumulate in persistent PSUM over all 64 tiles
  - final: phi *= 1/s, DMA out.

v3 scheduling (from the v2 trace): everything rides ONE HWDGE ring
(sync) in strict FIFO priority order [wt, xt0, xt1, setup-blob,
xn0a.., interleaved xt/xn chunks] so the startup-critical wt/xt bytes
are never starved by the 16 MB xn stream (v2 lost ~35 us to that).
All small consts are packed into one blob DMA (one descriptor instead
of eight ~650 ns descriptor-gens on the SP queue). The U_comb setup
matmuls are emitted after We(2) so the PE starts on real We work as
soon as wt+xt0 land; chain(0..2) emission is deferred until after
setup so the ucomb write precedes its readers in program order.
"""

from contextlib import ExitStack

import numpy as np
import ml_dtypes

import concourse.bacc as bacc
import concourse.tile as tile
from concourse.tile import add_dep_helper
from concourse import mybir
from concourse.bass_utils import run_bass_kernel_spmd

B = 8

BF = mybir.dt.bfloat16
F32 = mybir.dt.float32
NPBF = ml_dtypes.bfloat16

T, N, D, H = 128, 64, 1024, 512
BTN = T * N            # 8192
NI = BTN // 128        # 64 btn-tiles of 128 rows
KT = D // 128          # 8 contraction tiles over d
NCH = 8                # DMA chunks over btn-tiles
CPT = NI // NCH        # 8 tiles per chunk
PHI_LAG = 3            # tiles between chain(i) and phi(i) in PE order

# blob column offsets (bf16 [128, BLOB_C]); row-0 tail holds bvec/onesrow
O_HIDT = 0                      # [128, KT*128]  hidT with n-columns duplicated
O_UT = O_HIDT + KT * 128        # [128, KT*512]  U^T tiles
O_WREP = O_UT + KT * H          # [128, 512]     w replicated over partitions
O_BASE = O_WREP + H             # [128, 254]     base_log band
O_ONEC = O_BASE + 254           # [128, 1]       ones column
O_BVEC = O_ONEC + 1             # [1, 512]       Wb+Ub (row 0)
O_ONR = O_BVEC + H              # [1, 128]       ones row (row 0)
BLOB_C = O_ONR + 128


def build(nc):
    xt_d = nc.dram_tensor("xt", [128, NI * KT * 128], BF, kind="ExternalInput").ap()
    xn_d = nc.dram_tensor("xn", [128, NI * D], BF, kind="ExternalInput").ap()
    wt_d = nc.dram_tensor("wt", [128, KT * H], BF, kind="ExternalInput").ap()
    blob_d = nc.dram_tensor("blob", [128, BLOB_C], BF, kind="ExternalInput").ap()
    phi_d = nc.dram_tensor("phi", [T, D], F32, kind="ExternalOutput").ap()

    with tile.TileContext(nc) as tc, ExitStack() as ctx:
        consts = ctx.enter_context(tc.tile_pool(name="consts", bufs=1))
        xtp = ctx.enter_context(tc.tile_pool(name="xt", bufs=3))
        xnp = ctx.enter_context(tc.tile_pool(name="xn", bufs=4))
        attp = ctx.enter_context(tc.tile_pool(name="att", bufs=3))
        smal = ctx.enter_context(tc.tile_pool(name="smalls", bufs=6))
        pswe = ctx.enter_context(tc.tile_pool(name="pswe", bufs=4, space="PSUM"))
        psph = ctx.enter_context(tc.tile_pool(name="psphi", bufs=1, space="PSUM"))
        pssm = ctx.enter_context(tc.tile_pool(name="pssml", bufs=1, space="PSUM"))

        # ---- single FIFO ring, priority order ----
        wt = consts.tile([128, KT, H], BF)
        nc.sync.dma_start(out=wt, in_=wt_d.rearrange("p (k h) -> p k h", k=KT))

        def emit_xt(c, dep=None):
            xtc = xtp.tile([128, CPT, KT * 128], BF, tag="xt")
            cw = CPT * KT * 128
            i = nc.sync.dma_start(out=xtc, in_=xt_d[:, c * cw : (c + 1) * cw])
            if dep is not None:
                add_dep_helper(i.ins, dep.ins, reason="stage x stream behind PE progress")
            return xtc

        def emit_xn_half(xnc, c, half, dep=None):
            h, cw = CPT // 2, CPT * D
            lo = c * cw + half * (cw // 2)
            i = nc.sync.dma_start(
                out=xnc[:, half * h : (half + 1) * h, :],
                in_=xn_d[:, lo : lo + cw // 2],
            )
            if dep is not None:
                add_dep_helper(i.ins, dep.ins, reason="stage x stream behind PE progress")
            return xnc

        def emit_xn(c, dep=None):
            xnc = xnp.tile([128, CPT, D], BF, tag="xn")
            emit_xn_half(xnc, c, 0, dep)
            emit_xn_half(xnc, c, 1, dep)
            return xnc

        # Startup: per-engine descriptor queues are FIFO and bandwidth-
        # bound (27 GB/s per engine), so order transfers by first need and
        # keep chunk 0 per-tile so We(0) starts after wt + 0.25 MB.
        xn_bufs = {}
        xtc0 = xtp.tile([128, CPT, KT * 128], BF, tag="xt")
        xt_bufs = {0: xtc0}
        tw = KT * 128

        def emit_xt0_tile(j):
            nc.sync.dma_start(out=xtc0[:, j, :], in_=xt_d[:, j * tw : (j + 1) * tw])

        blob = consts.tile([128, BLOB_C], BF)
        scratch = consts.tile([128, H], BF)  # HAM warmup fuel
        nc.gpsimd.memset(scratch, 0.0)
        emit_xt0_tile(0)
        emit_xt0_tile(1)

        hidT = blob[:, O_HIDT : O_HIDT + KT * 128].rearrange("p (k n) -> p k n", k=KT)
        ut = blob[:, O_UT : O_UT + KT * H].rearrange("p (k h) -> p k h", k=KT)
        w_rep = blob[:, O_WREP : O_WREP + H]
        base_log = blob[:, O_BASE : O_BASE + 254]
        onescol = blob[:, O_ONEC : O_ONEC + 1]
        bvec = blob[0:1, O_BVEC : O_BVEC + H]
        onesrow = blob[0:1, O_ONR : O_ONR + 128]

        # ---- persistent accumulators ----
        ps_phi0 = psph.tile([T, 512], F32, tag="phi0")
        ps_phi1 = psph.tile([T, 512], F32, tag="phi1")
        ps_phi = [ps_phi0, ps_phi1]
        ps_s = psph.tile([T, 1], F32, tag="s")

        ucomb = consts.tile([128, H], BF)

        def emit_setup():
            # U_comb[c, h] = hid[c%64] @ U^T + (Wb + Ub): hidT arrives with
            # its n-columns pre-duplicated to 128, so this lands directly in
            # the [128, 512] per-tile layout.
            ps_u = pssm.tile([128, H], F32, tag="u")
            nc.tensor.matmul(ps_u, lhsT=onesrow, rhs=bvec, start=True, stop=False)
            for kt in range(KT):
                nc.tensor.matmul(
                    ps_u, lhsT=hidT[:, kt, :], rhs=ut[:, kt, :],
                    start=False, stop=(kt == KT - 1),
                )
            nc.scalar.activation(ucomb, ps_u, mybir.ActivationFunctionType.Copy)

        def emit_we(ig, xtc):
            j = ig % CPT
            ps = pswe.tile([128, H], F32, tag="we")
            for kt in range(KT):
                m = nc.tensor.matmul(
                    ps,
                    lhsT=xtc[:, j, kt * 128 : (kt + 1) * 128],
                    rhs=wt[:, kt, :],
                    start=(kt == 0),
                    stop=(kt == KT - 1),
                )
            return ps, m

        def emit_chain(ig, ps):
            nc.vector.tensor_tensor(out=ps, in0=ps, in1=ucomb, op=mybir.AluOpType.add)
            att = attp.tile([128, H], BF, tag="att")
            nc.scalar.activation(att, ps, mybir.ActivationFunctionType.Tanh)
            scr = attp.tile([128, H], BF, tag="scr")
            ecol = smal.tile([128, 1], F32, tag="ecol")
            nc.vector.scalar_tensor_tensor(
                out=scr, in0=att, scalar=1.0, in1=w_rep,
                op0=mybir.AluOpType.mult, op1=mybir.AluOpType.mult,
                accum_out=ecol,
            )
            adiag = smal.tile([128, 128], BF, tag="adiag")
            nc.scalar.activation(
                adiag,
                base_log[:, 126 - 2 * ig : 254 - 2 * ig],
                mybir.ActivationFunctionType.Exp,
                bias=ecol,
            )
            return adiag

        def emit_phi(item):
            ig, xnc, adiag = item
            j = ig % CPT
            for dh in range(2):
                nc.tensor.matmul(
                    ps_phi[dh],
                    lhsT=adiag,
                    rhs=xnc[:, j, dh * 512 : (dh + 1) * 512],
                    start=(ig == 0),
                    stop=(ig == NI - 1),
                )
            nc.tensor.matmul(
                ps_s, lhsT=adiag, rhs=onescol, start=(ig == 0), stop=(ig == NI - 1)
            )

        # ---- main pipeline ----
        # PE order: We(0) We(1) We(2) setup We(3).. with phi(i) trailing
        # chain(i) by PHI_LAG tiles. chain(0..2) emission is deferred until
        # after setup so the ucomb write precedes its readers.
        # HAM warmup: garbage matmuls keep the PE busy (and the clock
        # gate at K=8/8) while the first wt/xt bytes stream in.
        ps_warm = pswe.tile([128, H], F32, tag="we")
        for _ in range(9):
            m_warm = nc.tensor.matmul(ps_warm, lhsT=scratch[:, 0:128], rhs=scratch,
                                      start=True, stop=True)
        nc.sync.dma_start(out=blob, in_=blob_d)
        for j in range(2, CPT):
            emit_xt0_tile(j)
        we_pend = []   # (ig, ps) awaiting chain emission
        phi_pend = []  # (ig, xnc, adiag) awaiting phi emission
        for ig in range(NI):
            c, j = ig // CPT, ig % CPT
            if j == 0 and 1 <= c and c + 2 <= NCH - 1:
                # chunks 3+: pool-rotation WAR sems already pace these
                xt_bufs[c + 2] = emit_xt(c + 2)
                xn_bufs[c + 2] = emit_xn(c + 2)
            ps, m_we = emit_we(ig, xt_bufs[c])
            # stage the early x stream behind PE progress so transfer
            # completion order matches need order
            if ig == 0:
                xnc0 = xnp.tile([128, CPT, D], BF, tag="xn")
                xn_bufs[0] = xnc0
                emit_xn_half(xn_bufs[0], 0, 0, m_we)
            elif ig == 1:
                emit_xn_half(xn_bufs[0], 0, 1, m_we)
                xt_bufs[1] = emit_xt(1, m_we)
            elif ig == 2:
                xnc1 = xnp.tile([128, CPT, D], BF, tag="xn")
                xn_bufs[1] = xnc1
                emit_xn_half(xn_bufs[1], 1, 0, m_we)
            elif ig == 3:
                emit_xn_half(xn_bufs[1], 1, 1, m_we)
            elif ig == 4:
                xt_bufs[2] = emit_xt(2, m_we)
                xnc2 = xnp.tile([128, CPT, D], BF, tag="xn")
                xn_bufs[2] = xnc2
                emit_xn_half(xn_bufs[2], 2, 0, m_we)
            elif ig == 5:
                emit_xn_half(xn_bufs[2], 2, 1, m_we)
            we_pend.append((ig, ps))
            if ig == 2:
                emit_setup()
            if ig >= 2:
                while we_pend:
                    wig, wps = we_pend.pop(0)
                    phi_pend.append((wig, xn_bufs[wig // CPT], emit_chain(wig, wps)))
                maxlag = 6 if ig < 16 else (PHI_LAG if ig < NI - 4 else 1)
                while len(phi_pend) > maxlag:
                    emit_phi(phi_pend.pop(0))
        for item in phi_pend:
            emit_phi(item)

        # ---- finalize: phi = ps_phi * (1/s_t) ----
        recip = smal.tile([128, 1], F32, tag="recip")
        nc.vector.reciprocal(recip, ps_s)
        phi_sb = consts.tile([T, D], F32)
        # the two 1/s scales run on different engines so they overlap
        nc.vector.tensor_scalar_mul(phi_sb[:, 0:512], ps_phi[0], recip)
        nc.scalar.activation(
            phi_sb[:, 512:1024], ps_phi[1],
            mybir.ActivationFunctionType.Copy, scale=recip,
        )
        for dh in range(2):
            nc.sync.dma_start(
                out=phi_d[:, dh * 512 : (dh + 1) * 512],
                in_=phi_sb[:, dh * 512 : (dh + 1) * 512],
            )

    return nc


def prep_consts(W_weight, W_bias, U_weight, U_bias, w_weight):
    def pack_T(M):  # [H, D] -> [128, KT*H] bf16, [p, kt*H + h] = M[h, kt*128+p]
        arr = M.T.astype(np.float32).reshape(KT, 128, H).transpose(1, 0, 2)
        return np.ascontiguousarray(arr.reshape(128, KT * H)).astype(NPBF)

    blob = np.zeros((128, BLOB_C), np.float32)
    # hidT filled per-core in prep_in_maps
    blob[:, O_UT : O_UT + KT * H] = pack_T(U_weight).astype(np.float32)
    blob[:, O_WREP : O_WREP + H] = w_weight[0][None, :]
    blob[:, O_BASE : O_BASE + 254] = -30000.0
    for p in range(128):
        blob[p, O_BASE + 126 + p // 64] = 0.0
    blob[:, O_ONEC] = 1.0
    blob[0, O_BVEC : O_BVEC + H] = W_bias + U_bias
    blob[0, O_ONR : O_ONR + 128] = 1.0
    return {"wt": pack_T(W_weight), "_blob_f32": blob}


_NC_CACHE = {}


def make_nc(num_devices=B):
    if num_devices not in _NC_CACHE:
        nc = bacc.Bacc(
            "TRN2", target_bir_lowering=False, debug=False, num_devices=num_devices
        )
        build(nc)
        nc.compile()
        _NC_CACHE[num_devices] = nc
    return _NC_CACHE[num_devices]


def prep_in_maps(img_features, hidden_state, consts):
    maps = []
    for b in range(B):
        xb = np.asarray(img_features[b], dtype=np.float32).reshape(BTN, D).astype(NPBF)
        xn = np.ascontiguousarray(
            xb.reshape(NI, 128, D).transpose(1, 0, 2)
        ).reshape(128, NI * D)
        xt = np.ascontiguousarray(
            xb.reshape(NI, 128, KT, 128).transpose(3, 0, 2, 1)
        ).reshape(128, NI * KT * 128)
        hb = np.asarray(hidden_state[:, b, :], dtype=np.float32)
        blob = consts["_blob_f32"].copy()
        # hidT[p, kt, c] = hid[c % 64, kt*128 + p] (n-columns duplicated)
        ht = hb.T.reshape(KT, 128, N).transpose(1, 0, 2)  # [p, kt, n]
        blob[:, O_HIDT : O_HIDT + KT * 128] = np.concatenate(
            [ht, ht], axis=2
        ).reshape(128, KT * 128)
        maps.append(
            {"xt": xt, "xn": xn, "wt": consts["wt"], "blob": blob.astype(NPBF)}
        )
    return maps


def run(inputs, trace=False, tmpdir=None):
    """Run the SPMD kernel; returns (phi [B,T,D] fp32, BassKernelResults)."""
    inputs = {k: np.asarray(v) for k, v in inputs.items()}
    consts = prep_consts(
        inputs["W_weight"], inputs["W_bias"], inputs["U_weight"], inputs["U_bias"],
        inputs["w_weight"],
    )
    in_maps = prep_in_maps(inputs["img_features"], inputs["hidden_state"], consts)
    nc = make_nc(B)
    last_err = None
    for attempt in range(3):
        try:
            res = run_bass_kernel_spmd(
                nc, in_maps, core_ids=list(range(B)), trace=trace, tmpdir=tmpdir
            )
            break
        except Exception as e:  # transient NRT_EXEC_UNIT_UNRECOVERABLE etc.
            last_err = e
            if "UNRECOVERABLE" not in str(e) and "UNAVAILABLE" not in str(e):
                raise
    else:
        raise last_err
    phi = np.stack([res.results[b]["phi"] for b in range(B)]).astype(np.float32)
    return phi, res


def kernel(**inputs) -> np.ndarray:
    phi, _ = run(inputs, trace=False)
    return phi



# revision 2
# speedup vs baseline: 1.2720x; 1.2720x over previous
"""Trainium2 Bass kernel for nn_Attention_53334903882008 (additive attention), v10.

Reference (per batch b):
  We  = img @ W^T + Wb;  Ue = (hid @ U^T + Ub) broadcast over T
  att = tanh(We + Ue);   e = att @ w + wb
  alpha = softmax_N(e);  phi = sum_n alpha * img      -> [B, T, D]

Sharding: data-parallel over B=8, one batch per NeuronCore; weights
replicated.

v10 over v9 (which ran at the bf16 PE roofline, 170 us):
  - The We matmul (8192x1024x512 per core = 64% of PE cycles) now runs
    6 of its 8 k-tiles in fp8-e4m3 with perf_mode=DoubleRow (2 fp8
    MACs/cell/cycle): 3 double-row MMs + 2 bf16 MMs per btn-tile
    instead of 8 bf16 MMs. W is pre-scaled by 16 host-side so its
    ~N(0, 1/1024) entries stay out of the e4m3 subnormal range; the
    1/16 descale is folded into the existing DVE scalar_tensor_tensor
    that adds U_comb. Simulated end-to-end rel err 0.0168 (gate 2e-2);
    phi and the U path stay bf16 (full-fp8 phi would be 0.033).
  - Two HWDGE rings instead of one: the scalar ring carries the
    stationary-side stream (wt8/wtb, xt chunks), the sync ring carries
    blob + the 16 MB xn stream + the phi output. On one ring the xn
    bulk paced the whole kernel (96% DMA occupancy, 233 GB/s); split,
    each ring has slack and the x-stream staging deps are unnecessary
    (pool-rotation WAR sems pace everything).
  - Longer PE warmup (garbage MMs on a zero scratch tile) so the HAM
    clock gate (4/8 -> 8/8 after ~3.4 us of sustained busy) fires
    during the DMA spin-up instead of ~20 us into the kernel.

Per 128-row btn-tile (64 tiles):
  - We[btn, h]*16 = sum_g xt8^T @ wt8 (DoubleRow, kt 0..5)
                  + sum_k xtb^T @ wtb (bf16, kt 6..7), PSUM f32
  - DVE stt: ps = ps*(1/16) + U_comb; tanh on ACT -> att bf16
  - e column [128,1] via one fused DVE scalar_tensor_tensor
    (out = att * w_rep, accum_out = sum_h)
  - adiag = exp(base_log + e) in ONE ACT op (bias = e per-partition;
    base_log is 0 on the block-diag band, -30000 off-band) -> the
    block-diagonal unnormalized-softmax matrix directly
  - phi[t,d] += adiag^T @ xn (2 bf16 matmuls) and s[t] += adiag^T @
    ones (N=1 matmul) accumulate in persistent PSUM over all 64 tiles
  - final: phi *= 1/s, DMA out.
"""

from contextlib import ExitStack

import numpy as np
import ml_dtypes

import concourse.bacc as bacc
import concourse.tile as tile
from concourse import mybir
from concourse.bass_utils import run_bass_kernel_spmd

B = 8

BF = mybir.dt.bfloat16
F8 = mybir.dt.float8e4
F32 = mybir.dt.float32
NPBF = ml_dtypes.bfloat16
NPF8 = ml_dtypes.float8_e4m3
DR = mybir.MatmulPerfMode.DoubleRow

T, N, D, H = 128, 64, 1024, 512
BTN = T * N            # 8192
NI = BTN // 128        # 64 btn-tiles of 128 rows
KT = D // 128          # 8 contraction tiles over d
G8 = 3                 # DoubleRow double-k groups (kt 0..5 in fp8)
NKTB = KT - 2 * G8     # trailing bf16 k-tiles (kt 6..7)
WS = 16.0              # host-side W scale (keeps W out of e4m3 subnormals)
NCH = 8                # DMA chunks over btn-tiles
CPT = NI // NCH        # 8 tiles per chunk
PHI_LAG = 3            # tiles between chain(i) and phi(i) in PE order
N_WARM = 12            # garbage warmup MMs (HAM warm + DMA spin-up cover)

X8C = G8 * 2 * 128     # 768 fp8 cols per tile in xt8
XBC = NKTB * 128       # 256 bf16 cols per tile in xtb

# blob column offsets (bf16 [128, BLOB_C]); row-0 tail holds bvec/onesrow
O_HIDT = 0                      # [128, KT*128]  hidT with n-columns duplicated
O_UT = O_HIDT + KT * 128        # [128, KT*512]  U^T tiles
O_WREP = O_UT + KT * H          # [128, 512]     w replicated over partitions
O_BASE = O_WREP + H             # [128, 254]     base_log band
O_ONEC = O_BASE + 254           # [128, 1]       ones column
O_BVEC = O_ONEC + 1             # [1, 512]       Wb+Ub (row 0)
O_ONR = O_BVEC + H              # [1, 128]       ones row (row 0)
BLOB_C = O_ONR + 128


def build(nc):
    xt8_d = nc.dram_tensor("xt8", [128, NI * X8C], F8, kind="ExternalInput").ap()
    xtb_d = nc.dram_tensor("xtb", [128, NI * XBC], BF, kind="ExternalInput").ap()
    xn_d = nc.dram_tensor("xn", [128, NI * D], BF, kind="ExternalInput").ap()
    wt8_d = nc.dram_tensor("wt8", [128, G8 * 2 * H], F8, kind="ExternalInput").ap()
    wtb_d = nc.dram_tensor("wtb", [128, NKTB * H], BF, kind="ExternalInput").ap()
    blob_d = nc.dram_tensor("blob", [128, BLOB_C], BF, kind="ExternalInput").ap()
    phi_d = nc.dram_tensor("phi", [T, D], F32, kind="ExternalOutput").ap()

    with tile.TileContext(nc) as tc, ExitStack() as ctx:
        consts = ctx.enter_context(tc.tile_pool(name="consts", bufs=1))
        xtp = ctx.enter_context(tc.tile_pool(name="xt", bufs=3))
        xnp = ctx.enter_context(tc.tile_pool(name="xn", bufs=4))
        attp = ctx.enter_context(tc.tile_pool(name="att", bufs=3))
        smal = ctx.enter_context(tc.tile_pool(name="smalls", bufs=6))
        pswe = ctx.enter_context(tc.tile_pool(name="pswe", bufs=4, space="PSUM"))
        psph = ctx.enter_context(tc.tile_pool(name="psphi", bufs=1, space="PSUM"))
        pssm = ctx.enter_context(tc.tile_pool(name="pssml", bufs=1, space="PSUM"))

        # ---- scalar ring: stationary-side stream (wt, xt chunks) ----
        wt8 = consts.tile([128, G8, 2, H], F8)
        nc.scalar.dma_start(out=wt8, in_=wt8_d.rearrange("p (g i h) -> p g i h", g=G8, i=2))
        wtb = consts.tile([128, NKTB, H], BF)
        nc.scalar.dma_start(out=wtb, in_=wtb_d.rearrange("p (k h) -> p k h", k=NKTB))

        def emit_xt(c):
            xt8c = xtp.tile([128, CPT, G8, 2, 128], F8, tag="xt8")
            xtbc = xtp.tile([128, CPT, NKTB, 128], BF, tag="xtb")
            c8, cb = CPT * X8C, CPT * XBC
            nc.scalar.dma_start(out=xt8c, in_=xt8_d[:, c * c8 : (c + 1) * c8])
            nc.scalar.dma_start(out=xtbc, in_=xtb_d[:, c * cb : (c + 1) * cb])
            return xt8c, xtbc

        def emit_xn(c):
            xnc = xnp.tile([128, CPT, D], BF, tag="xn")
            h, cw = CPT // 2, CPT * D
            for half in range(2):
                lo = c * cw + half * (cw // 2)
                nc.sync.dma_start(
                    out=xnc[:, half * h : (half + 1) * h, :],
                    in_=xn_d[:, lo : lo + cw // 2],
                )
            return xnc

        # Startup: chunk 0 in 2-tile pieces so We(0) starts after
        # wt + ~0.3 MB instead of a full 1 MB chunk.
        xt8c0 = xtp.tile([128, CPT, G8, 2, 128], F8, tag="xt8")
        xtbc0 = xtp.tile([128, CPT, NKTB, 128], BF, tag="xtb")
        for q in range(4):
            nc.scalar.dma_start(
                out=xt8c0[:, 2 * q : 2 * q + 2],
                in_=xt8_d[:, 2 * q * X8C : (2 * q + 2) * X8C],
            )
            nc.scalar.dma_start(
                out=xtbc0[:, 2 * q : 2 * q + 2],
                in_=xtb_d[:, 2 * q * XBC : (2 * q + 2) * XBC],
            )
        xt_bufs = {0: (xt8c0, xtbc0)}
        xn_bufs = {}

        # ---- sync ring: blob first, then the xn bulk stream ----
        blob = consts.tile([128, BLOB_C], BF)
        nc.sync.dma_start(out=blob, in_=blob_d)
        scratch = consts.tile([128, H], BF)  # HAM warmup fuel
        nc.gpsimd.memset(scratch, 0.0)

        hidT = blob[:, O_HIDT : O_HIDT + KT * 128].rearrange("p (k n) -> p k n", k=KT)
        ut = blob[:, O_UT : O_UT + KT * H].rearrange("p (k h) -> p k h", k=KT)
        w_rep = blob[:, O_WREP : O_WREP + H]
        base_log = blob[:, O_BASE : O_BASE + 254]
        onescol = blob[:, O_ONEC : O_ONEC + 1]
        bvec = blob[0:1, O_BVEC : O_BVEC + H]
        onesrow = blob[0:1, O_ONR : O_ONR + 128]

        # ---- persistent accumulators ----
        ps_phi0 = psph.tile([T, 512], F32, tag="phi0")
        ps_phi1 = psph.tile([T, 512], F32, tag="phi1")
        ps_phi = [ps_phi0, ps_phi1]
        ps_s = psph.tile([T, 1], F32, tag="s")

        ucomb = consts.tile([128, H], BF)

        def emit_setup():
            # U_comb[c, h] = hid[c%64] @ U^T + (Wb + Ub): hidT arrives with
            # its n-columns pre-duplicated to 128, so this lands directly in
            # the [128, 512] per-tile layout.
            ps_u = pssm.tile([128, H], F32, tag="u")
            nc.tensor.matmul(ps_u, lhsT=onesrow, rhs=bvec, start=True, stop=False)
            for kt in range(KT):
                nc.tensor.matmul(
                    ps_u, lhsT=hidT[:, kt, :], rhs=ut[:, kt, :],
                    start=False, stop=(kt == KT - 1),
                )
            nc.scalar.activation(ucomb, ps_u, mybir.ActivationFunctionType.Copy)

        def emit_we(ig, bufs):
            xt8c, xtbc = bufs
            j = ig % CPT
            ps = pswe.tile([128, H], F32, tag="we")
            for g in range(G8):
                nc.tensor.matmul(
                    ps, lhsT=xt8c[:, j, g, :, :], rhs=wt8[:, g, :, :],
                    start=(g == 0), stop=False, perf_mode=DR,
                )
            for k in range(NKTB):
                m = nc.tensor.matmul(
                    ps, lhsT=xtbc[:, j, k, :], rhs=wtb[:, k, :],
                    start=False, stop=(k == NKTB - 1),
                )
            return ps, m

        def emit_chain(ig, ps):
            # ps holds 16*We; descale and add U_comb in one DVE op.
            nc.vector.scalar_tensor_tensor(
                out=ps, in0=ps, scalar=1.0 / WS, in1=ucomb,
                op0=mybir.AluOpType.mult, op1=mybir.AluOpType.add,
            )
            att = attp.tile([128, H], BF, tag="att")
            nc.scalar.activation(att, ps, mybir.ActivationFunctionType.Tanh)
            scr = attp.tile([128, H], BF, tag="scr")
            ecol = smal.tile([128, 1], F32, tag="ecol")
            nc.vector.scalar_tensor_tensor(
                out=scr, in0=att, scalar=1.0, in1=w_rep,
                op0=mybir.AluOpType.mult, op1=mybir.AluOpType.mult,
                accum_out=ecol,
            )
            adiag = smal.tile([128, 128], BF, tag="adiag")
            nc.scalar.activation(
                adiag,
                base_log[:, 126 - 2 * ig : 254 - 2 * ig],
                mybir.ActivationFunctionType.Exp,
                bias=ecol,
            )
            return adiag

        def emit_phi(item):
            ig, xnc, adiag = item
            j = ig % CPT
            for dh in range(2):
                nc.tensor.matmul(
                    ps_phi[dh],
                    lhsT=adiag,
                    rhs=xnc[:, j, dh * 512 : (dh + 1) * 512],
                    start=(ig == 0), stop=(ig == NI - 1),
                )
            nc.tensor.matmul(
                ps_s, lhsT=adiag, rhs=onescol, start=(ig == 0), stop=(ig == NI - 1)
            )

        # ---- main pipeline ----
        # PE order: warmup, We(0) We(1) We(2) setup We(3).. with phi(i)
        # trailing chain(i) by PHI_LAG tiles. chain(0..2) emission is
        # deferred until after setup so the ucomb write precedes its
        # readers in program order. Warmup garbage MMs keep the PE busy
        # (and the HAM clock gate moving toward 8/8) while the first
        # wt/xt bytes stream in.
        ps_warm = pswe.tile([128, H], F32, tag="we")
        for _ in range(N_WARM):
            nc.tensor.matmul(ps_warm, lhsT=scratch[:, 0:128], rhs=scratch,
                             start=True, stop=True)
        # early xn chunks (sync ring, behind blob; pool WAR paces the rest)
        xn_bufs[0] = emit_xn(0)
        xn_bufs[1] = emit_xn(1)
        xn_bufs[2] = emit_xn(2)
        we_pend = []   # (ig, ps) awaiting chain emission
        phi_pend = []  # (ig, xnc, adiag) awaiting phi emission
        for ig in range(NI):
            c, j = ig // CPT, ig % CPT
            if j == 0 and 1 <= c and c + 2 <= NCH - 1:
                xt_bufs[c + 2] = emit_xt(c + 2)
                xn_bufs[c + 2] = emit_xn(c + 2)
            if ig == 1:
                xt_bufs[1] = emit_xt(1)
            elif ig == 4:
                xt_bufs[2] = emit_xt(2)
            ps, m_we = emit_we(ig, xt_bufs[c])
            we_pend.append((ig, ps))
            if ig == 2:
                emit_setup()
            if ig >= 2:
                while we_pend:
                    wig, wps = we_pend.pop(0)
                    phi_pend.append((wig, xn_bufs[wig // CPT], emit_chain(wig, wps)))
                maxlag = 6 if ig < 16 else (PHI_LAG if ig < NI - 4 else 1)
                while len(phi_pend) > maxlag:
                    emit_phi(phi_pend.pop(0))
        for item in phi_pend:
            emit_phi(item)

        # ---- finalize: phi = ps_phi * (1/s_t) ----
        recip = smal.tile([128, 1], F32, tag="recip")
        nc.vector.reciprocal(recip, ps_s)
        phi_sb = consts.tile([T, D], F32)
        # the two 1/s scales run on different engines so they overlap
        nc.vector.tensor_scalar_mul(phi_sb[:, 0:512], ps_phi[0], recip)
        nc.scalar.activation(
            phi_sb[:, 512:1024], ps_phi[1],
            mybir.ActivationFunctionType.Copy, scale=recip,
        )
        for dh in range(2):
            nc.sync.dma_start(
                out=phi_d[:, dh * 512 : (dh + 1) * 512],
                in_=phi_sb[:, dh * 512 : (dh + 1) * 512],
            )

    return nc


def prep_consts(W_weight, W_bias, U_weight, U_bias, w_weight):
    def pack_T(M):  # [H, D] -> [128, KT, H] f32, [p, kt, h] = M[h, kt*128+p]
        return M.T.astype(np.float32).reshape(KT, 128, H).transpose(1, 0, 2)

    wkt = pack_T(W_weight) * WS            # [128, KT, H], scaled
    wt8 = np.ascontiguousarray(wkt[:, : 2 * G8, :]).reshape(128, G8 * 2 * H)
    wt8 = np.clip(wt8, -240, 240).astype(NPF8)
    wtb = np.ascontiguousarray(wkt[:, 2 * G8 :, :]).reshape(128, NKTB * H).astype(NPBF)

    ukt = pack_T(U_weight)
    blob = np.zeros((128, BLOB_C), np.float32)
    # hidT filled per-core in prep_in_maps
    blob[:, O_UT : O_UT + KT * H] = ukt.reshape(128, KT * H)
    blob[:, O_WREP : O_WREP + H] = w_weight[0][None, :]
    blob[:, O_BASE : O_BASE + 254] = -30000.0
    for p in range(128):
        blob[p, O_BASE + 126 + p // 64] = 0.0
    blob[:, O_ONEC] = 1.0
    blob[0, O_BVEC : O_BVEC + H] = W_bias + U_bias
    blob[0, O_ONR : O_ONR + 128] = 1.0
    return {"wt8": wt8, "wtb": wtb, "_blob_f32": blob}


_NC_CACHE = {}


def make_nc(num_devices=B):
    if num_devices not in _NC_CACHE:
        nc = bacc.Bacc(
            "TRN2", target_bir_lowering=False, debug=False, num_devices=num_devices
        )
        build(nc)
        nc.compile()
        _NC_CACHE[num_devices] = nc
    return _NC_CACHE[num_devices]


def prep_in_maps(img_features, hidden_state, consts):
    maps = []
    for b in range(B):
        xb = np.asarray(img_features[b], dtype=np.float32).reshape(BTN, D)
        xn = np.ascontiguousarray(
            xb.astype(NPBF).reshape(NI, 128, D).transpose(1, 0, 2)
        ).reshape(128, NI * D)
        # xkt[c-in-tile, tile, kt, p] views for the stationary stream
        xkt = xb.reshape(NI, 128, KT, 128)
        xt8 = np.ascontiguousarray(
            xkt[:, :, : 2 * G8, :].transpose(3, 0, 2, 1)  # [p, tile, kt, c]
        ).reshape(128, NI * X8C)
        xt8 = np.clip(xt8, -240, 240).astype(NPF8)
        xtb = np.ascontiguousarray(
            xkt[:, :, 2 * G8 :, :].transpose(3, 0, 2, 1)
        ).reshape(128, NI * XBC).astype(NPBF)
        hb = np.asarray(hidden_state[:, b, :], dtype=np.float32)
        blob = consts["_blob_f32"].copy()
        # hidT[p, kt, c] = hid[c % 64, kt*128 + p] (n-columns duplicated)
        ht = hb.T.reshape(KT, 128, N).transpose(1, 0, 2)  # [p, kt, n]
        blob[:, O_HIDT : O_HIDT + KT * 128] = np.concatenate(
            [ht, ht], axis=2
        ).reshape(128, KT * 128)
        maps.append(
            {
                "xt8": xt8, "xtb": xtb, "xn": xn,
                "wt8": consts["wt8"], "wtb": consts["wtb"],
                "blob": blob.astype(NPBF),
            }
        )
    return maps


def run(inputs, trace=False, tmpdir=None):
    """Run the SPMD kernel; returns (phi [B,T,D] fp32, BassKernelResults)."""
    inputs = {k: np.asarray(v) for k, v in inputs.items()}
    consts = prep_consts(
        inputs["W_weight"], inputs["W_bias"], inputs["U_weight"], inputs["U_bias"],
        inputs["w_weight"],
    )
    in_maps = prep_in_maps(inputs["img_features"], inputs["hidden_state"], consts)
    nc = make_nc(B)
    last_err = None
    for attempt in range(3):
        try:
            res = run_bass_kernel_spmd(
                nc, in_maps, core_ids=list(range(B)), trace=trace, tmpdir=tmpdir
            )
            break
        except Exception as e:  # transient NRT_EXEC_UNIT_UNRECOVERABLE etc.
            last_err = e
            if "UNRECOVERABLE" not in str(e) and "UNAVAILABLE" not in str(e):
                raise
    else:
        raise last_err
    phi = np.stack([res.results[b]["phi"] for b in range(B)]).astype(np.float32)
    return phi, res


def kernel(**inputs) -> np.ndarray:
    phi, _ = run(inputs, trace=False)
    return phi
